# revision 4
# baseline (speedup 1.0000x reference)
"""GATv2 (2-layer + skips) on 8 Trainium2 NeuronCores.

Strategy (node-parallel with degree bucketing):
 - Host: sort nodes by in-degree, deal round-robin to 8 cores, tile each
   core's nodes into 49 groups of 128 with a shared per-tile padded
   neighbor count K_t.  All graph index/mask arrays are precomputed host-side
   (they are functions of edge_index only, i.e. sharding metadata).
 - Launch A: per-core dense matmuls xl1/xr1/skip1 from x.
 - Host: assemble the global xl1 table (+ zero row for padding slots).
 - Launch BC: per node tile, indirect-gather the K_t neighbor rows of xl1,
   compute GATv2 scores, masked segment softmax and the weighted
   aggregation entirely as dense row ops (no scatter), apply skip+relu to
   get h, then immediately compute xl2/xr2/skip2 = linear(h) on-chip.
 - Host: assemble the global xl2 table.
 - Launch D: same GAT pipeline for layer 2 -> final output rows.
 - Host: undo the node permutation.

Everything numerical runs on-device in f32; the host only shards, permutes
and concatenates.
"""

import sys
import types
import contextlib
import ctypes

sys.path.insert(0, "/opt/trn_rl_repo")

import numpy as np

import concourse.bacc as bacc
import concourse.bass as bass
import concourse.tile as tile
import concourse.mybir as mybir
from concourse.masks import make_identity
from concourse.bass_utils import run_bass_kernel_spmd

# ----------------------------------------------------------------------------
# axon NTFF profiling hook (the container image lacks antenv.axon_hooks)
# ----------------------------------------------------------------------------
_SO_PATH = "/opt/axon/libaxon_pjrt.so"


def _ntff_profile_via_ctypes(so_path):
    try:
        lib = ctypes.CDLL(so_path)
    except OSError:
        return None
    if not hasattr(lib, "axon_start_nrt_profile"):
        return None
    lib.axon_start_nrt_profile.argtypes = [ctypes.POINTER(ctypes.c_int64), ctypes.c_size_t]
    lib.axon_start_nrt_profile.restype = ctypes.c_int64
    lib.axon_stop_nrt_profile.argtypes = [ctypes.c_char_p]
    lib.axon_stop_nrt_profile.restype = ctypes.c_int64

    @contextlib.contextmanager
    def _hook(output_dir, device_ids):
        import jax

        jax.devices()
        if device_ids:
            ids = (ctypes.c_int64 * len(device_ids))(*device_ids)
            rc = lib.axon_start_nrt_profile(ids, len(device_ids))
        else:
            rc = lib.axon_start_nrt_profile(None, 0)
        if rc != 0:
            raise RuntimeError(f"axon_start_nrt_profile rc={rc}")
        try:
            yield
        finally:
            n = lib.axon_stop_nrt_profile(str(output_dir).encode())
            if n < 0:
                raise RuntimeError(f"axon_stop_nrt_profile rc={n}")

    return _hook


def _install_hooks():
    if "antenv.axon_hooks" not in sys.modules:
        m = types.ModuleType("antenv.axon_hooks")
        m._hook = None
        m.set_axon_ntff_profile_hook = lambda h: setattr(m, "_hook", h)
        m.get_axon_ntff_profile_hook = lambda: m._hook
        sys.modules["antenv.axon_hooks"] = m
    sys.modules["antenv.axon_hooks"].set_axon_ntff_profile_hook(
        _ntff_profile_via_ctypes(_SO_PATH)
    )
    from concourse import bass_utils

    bass_utils.upload_artifacts = lambda tmpdir: tmpdir


_install_hooks()

# ----------------------------------------------------------------------------
# problem constants (hardcoded per the task contract)
# ----------------------------------------------------------------------------
N_NODES = 50000
N_EDGES = 800000
D_IN = 128
HID = 128
OUT = 64
NEG_SLOPE = 0.2
C = 8            # cores
P = 128          # partitions
NEG_BIG = -1.0e9

F32 = mybir.dt.float32
I32 = mybir.dt.int32

# exec times of the launches from the most recent kernel() call
LAST_EXEC_NS = []
TRACE = True


# ----------------------------------------------------------------------------
# host-side preprocessing: sharding metadata from edge_index
# ----------------------------------------------------------------------------
def prep(edge_index, n_nodes=N_NODES, n_cores=C):
    src = np.asarray(edge_index[0]).astype(np.int64)
    dst = np.asarray(edge_index[1]).astype(np.int64)
    deg = np.bincount(dst, minlength=n_nodes).astype(np.int64)

    order = np.argsort(deg, kind="stable")          # nodes by in-degree asc
    per = n_nodes // n_cores
    npc = ((per + P - 1) // P) * P                  # nodes per core incl. dummies
    n_dummy = npc - per
    nt = npc // P                                   # tiles per core

    # dst-sorted CSR
    e_order = np.argsort(dst, kind="stable")
    srcs_sorted = src[e_order]
    row_start = np.zeros(n_nodes + 1, np.int64)
    np.cumsum(deg, out=row_start[1:])

    # per-core node lists (dummies first so they land in the low-K tiles)
    nodes_mat = np.full((n_cores, npc), -1, np.int64)
    for c in range(n_cores):
        nodes_mat[c, n_dummy:] = order[c::n_cores]

    # global position of each node in the assembled tables; zero row at the end
    nv = n_cores * npc + 1
    zrow = nv - 1
    pos = np.zeros(n_nodes, np.int64)
    for c in range(n_cores):
        pos[nodes_mat[c, n_dummy:]] = c * npc + n_dummy + np.arange(per)

    deg_pad = np.concatenate([deg, [0]])            # deg_pad[-1] for dummy -1

    # per-tile K (shared across cores so the program is uniform)
    Ks = []
    for t in range(nt):
        rows = nodes_mat[:, t * P : (t + 1) * P]
        Ks.append(max(1, int(deg_pad[rows].max())))

    # gather index + mask arrays, [sum_t 128*K_t] per core, tile-major
    tot = sum(Ks) * P
    idx_arr = np.empty((n_cores, tot), np.int32)
    mask_arr = np.empty((n_cores, tot), np.float32)
    off = 0
    for t in range(nt):
        K = Ks[t]
        rows = nodes_mat[:, t * P : (t + 1) * P]            # [C, 128]
        dr = deg_pad[rows]                                  # [C, 128]
        ks = np.arange(K)[None, None, :]                    # [1, 1, K]
        valid = ks < dr[:, :, None]                         # [C, 128, K]
        eidx = row_start[np.clip(rows, 0, None)][:, :, None] + ks
        eidx = np.clip(eidx, 0, src.shape[0] - 1)
        srcs = srcs_sorted[eidx]                            # [C, 128, K]
        vals = np.where(valid, pos[srcs], zrow).astype(np.int32)
        msk = np.where(valid, 0.0, NEG_BIG).astype(np.float32)
        idx_arr[:, off : off + P * K] = vals.reshape(n_cores, P * K)
        mask_arr[:, off : off + P * K] = msk.reshape(n_cores, P * K)
        off += P * K

    return dict(
        nodes_mat=nodes_mat, npc=npc, nt=nt, nv=nv, Ks=Ks,
        idx=idx_arr, mask=mask_arr, n_dummy=n_dummy, per=per,
    )


# ----------------------------------------------------------------------------
# device program builders
# ----------------------------------------------------------------------------
def _bias_bcast_ap(vec_ap, nparts=P):
    return bass.AP(tensor=vec_ap.tensor, offset=vec_ap.offset,
                   ap=[[0, nparts]] + list(vec_ap.ap))


def build_linear(npc, h_in, h_out, n_cores=C):
    """xsT [h_in, npc] -> xl/xr/skipb [npc, h_out] (3 matmuls + biases)."""
    nc = bacc.Bacc("TRN2", target_bir_lowering=False, debug=False, num_devices=n_cores)
    xsT = nc.dram_tensor("xsT", [h_in, npc], F32, kind="ExternalInput").ap()
    ws = {}
    for nm in ("wl", "wr", "ws"):
        ws[nm] = nc.dram_tensor(nm, [h_in, h_out], F32, kind="ExternalInput").ap()
    bs = {}
    for nm in ("bl", "br", "bsk"):
        bs[nm] = nc.dram_tensor(nm, [h_out], F32, kind="ExternalInput").ap()
    outs = {}
    for nm in ("xl", "xr", "skipb"):
        outs[nm] = nc.dram_tensor("o_" + nm, [npc, h_out], F32, kind="ExternalOutput").ap()

    nt = npc // P
    with tile.TileContext(nc) as tc:
        with (
            tc.tile_pool(name="consts", bufs=1) as consts,
            tc.tile_pool(name="work", bufs=3) as work,
            tc.tile_pool(name="ps", bufs=4, space="PSUM") as ps,
        ):
            w_t = {}
            b_t = {}
            for nm in ("wl", "wr", "ws"):
                w_t[nm] = consts.tile([h_in, h_out], F32, tag="w_" + nm, name="w_" + nm)
                nc.sync.dma_start(out=w_t[nm][:], in_=ws[nm][:, :])
            for nm in ("bl", "br", "bsk"):
                b_t[nm] = consts.tile([P, h_out], F32, tag="b_" + nm, name="b_" + nm)
                nc.gpsimd.dma_start(out=b_t[nm][:], in_=_bias_bcast_ap(bs[nm]))
            for t in range(nt):
                lhs = work.tile([h_in, P], F32, tag="lhs")
                nc.sync.dma_start(out=lhs[:], in_=xsT[:, t * P : (t + 1) * P])
                for nm, wnm, bnm in (("xl", "wl", "bl"), ("xr", "wr", "br"),
                                     ("skipb", "ws", "bsk")):
                    pt = ps.tile([P, h_out], F32, tag="mm")
                    nc.tensor.matmul(out=pt[:], lhsT=lhs[:], rhs=w_t[wnm][:],
                                     start=True, stop=True)
                    ot = work.tile([P, h_out], F32, tag="o_" + nm, name="o_" + nm)
                    nc.vector.tensor_tensor(out=ot[:], in0=pt[:], in1=b_t[bnm][:],
                                            op=mybir.AluOpType.add)
                    nc.sync.dma_start(out=outs[nm][t * P : (t + 1) * P, :], in_=ot[:])
    nc.compile()
    return nc


def build_gat(npc, nv, Ks, h, h2=None, n_cores=C, alpha=NEG_SLOPE):
    """One GAT layer over per-core node tiles.

    inputs: xlf [nv, h] (global xl table), xr/skipb [npc, h], idx/mask
    [sum 128*K_t], att [h].  If h2 is given, also computes the next layer's
    linear (wl2/wr2/ws2 [h, h2] + biases) from this layer's h output and
    emits xl/xr/skipb [npc, h2]; otherwise emits the layer output [npc, h].
    """
    nc = bacc.Bacc("TRN2", target_bir_lowering=False, debug=False, num_devices=n_cores)
    tot = sum(Ks) * P
    xlf = nc.dram_tensor("xlf", [nv, h], F32, kind="ExternalInput").ap()
    xr = nc.dram_tensor("xr", [npc, h], F32, kind="ExternalInput").ap()
    skipb = nc.dram_tensor("skipb", [npc, h], F32, kind="ExternalInput").ap()
    idx = nc.dram_tensor("idx", [tot], I32, kind="ExternalInput").ap()
    mask = nc.dram_tensor("mask", [tot], F32, kind="ExternalInput").ap()
    att = nc.dram_tensor("att", [h], F32, kind="ExternalInput").ap()
    if h2 is not None:
        ws = {}
        for nm in ("wl2", "wr2", "ws2"):
            ws[nm] = nc.dram_tensor(nm, [h, h2], F32, kind="ExternalInput").ap()
        bs = {}
        for nm in ("bl2", "br2", "bsk2"):
            bs[nm] = nc.dram_tensor(nm, [h2], F32, kind="ExternalInput").ap()
        outs = {}
        for nm in ("xl", "xr", "skipb"):
            outs[nm] = nc.dram_tensor("o_" + nm, [npc, h2], F32, kind="ExternalOutput").ap()
    else:
        hout = nc.dram_tensor("o_h", [npc, h], F32, kind="ExternalOutput").ap()

    Kmax = max(Ks)
    nt = npc // P
    ADD = mybir.AluOpType.add
    MULT = mybir.AluOpType.mult
    MAX = mybir.AluOpType.max

    with tile.TileContext(nc) as tc:
        with (
            tc.tile_pool(name="consts", bufs=1) as consts,
            tc.tile_pool(name="big", bufs=2) as big,
            tc.tile_pool(name="med", bufs=3) as med,
            tc.tile_pool(name="sm", bufs=3) as sm,
            tc.tile_pool(name="ps", bufs=2, space="PSUM") as ps,
        ):
            att_t = consts.tile([P, h], F32, tag="att")
            nc.gpsimd.dma_start(out=att_t[:], in_=_bias_bcast_ap(att))
            if h2 is not None:
                ident = consts.tile([P, P], F32, tag="ident")
                make_identity(nc, ident[:])
                w_t = {}
                b_t = {}
                for nm in ("wl2", "wr2", "ws2"):
                    w_t[nm] = consts.tile([h, h2], F32, tag="w_" + nm, name="w_" + nm)
                    nc.sync.dma_start(out=w_t[nm][:], in_=ws[nm][:, :])
                for nm in ("bl2", "br2", "bsk2"):
                    b_t[nm] = consts.tile([P, h2], F32, tag="b_" + nm, name="b_" + nm)
                    nc.gpsimd.dma_start(out=b_t[nm][:], in_=_bias_bcast_ap(bs[nm]))

            off = 0
            for t in range(nt):
                K = Ks[t]
                r0 = t * P
                idx_t = sm.tile([P, K], I32, tag="idx")
                nc.sync.dma_start(
                    out=idx_t[:],
                    in_=idx[off : off + P * K].rearrange("(p k) -> p k", k=K))
                mask_t = sm.tile([P, K], F32, tag="mask")
                nc.sync.dma_start(
                    out=mask_t[:],
                    in_=mask[off : off + P * K].rearrange("(p k) -> p k", k=K))
                off += P * K
                xr_t = med.tile([P, h], F32, tag="xr")
                nc.sync.dma_start(out=xr_t[:], in_=xr[r0 : r0 + P, :])
                skipb_t = med.tile([P, h], F32, tag="skipb")
                nc.sync.dma_start(out=skipb_t[:], in_=skipb[r0 : r0 + P, :])

                g = big.tile([P, K * h], F32, tag="g")
                for k in range(K):
                    nc.gpsimd.indirect_dma_start(
                        out=g[:, k * h : (k + 1) * h],
                        out_offset=None,
                        in_=xlf[:, :],
                        in_offset=bass.IndirectOffsetOnAxis(
                            ap=idx_t[:, k : k + 1], axis=0),
                    )

                g3 = g[:].rearrange("p (k h) -> p k h", k=K)
                u = big.tile([P, K * h], F32, tag="u")
                u3 = u[:].rearrange("p (k h) -> p k h", k=K)
                xr_b = xr_t[:].unsqueeze(1).to_broadcast([P, K, h])
                nc.vector.tensor_tensor(out=u3, in0=g3, in1=xr_b, op=ADD)
                # leaky_relu(u) = max(alpha*u, u) for 0 < alpha < 1
                nc.vector.scalar_tensor_tensor(
                    out=u[:], in0=u[:], scalar=alpha, in1=u[:], op0=MULT, op1=MAX)

                s_t = sm.tile([P, K], F32, tag="s")
                junk = med.tile([P, h], F32, tag="junk")
                for k in range(K):
                    nc.vector.scalar_tensor_tensor(
                        out=junk[:], in0=u[:, k * h : (k + 1) * h], scalar=1.0,
                        in1=att_t[:], op0=MULT, op1=MULT,
                        accum_out=s_t[:, k : k + 1])
                nc.vector.tensor_tensor(out=s_t[:], in0=s_t[:], in1=mask_t[:], op=ADD)
                negm = sm.tile([P, 1], F32, tag="negm")
                nc.vector.tensor_reduce(out=negm[:], in_=s_t[:],
                                        axis=mybir.AxisListType.X, op=MAX, negate=True)
                ex = sm.tile([P, K], F32, tag="ex")
                nc.scalar.activation(out=ex[:], in_=s_t[:],
                                     func=mybir.ActivationFunctionType.Exp,
                                     bias=negm[:], scale=1.0)
                ssum = sm.tile([P, 1], F32, tag="ssum")
                nc.vector.tensor_reduce(out=ssum[:], in_=ex[:],
                                        axis=mybir.AxisListType.X, op=ADD)
                rcp = sm.tile([P, 1], F32, tag="rcp")
                nc.vector.reciprocal(out=rcp[:], in_=ssum[:])

                agg = med.tile([P, h], F32, tag="agg")
                nc.vector.tensor_scalar(
                    out=agg[:], in0=g[:, 0:h], scalar1=ex[:, 0:1], scalar2=None,
                    op0=MULT)
                for k in range(1, K):
                    nc.vector.scalar_tensor_tensor(
                        out=agg[:], in0=g[:, k * h : (k + 1) * h],
                        scalar=ex[:, k : k + 1], in1=agg[:], op0=MULT, op1=ADD)

                h_t = med.tile([P, h], F32, tag="h")
                nc.vector.scalar_tensor_tensor(
                    out=h_t[:], in0=agg[:], scalar=rcp[:], in1=skipb_t[:],
                    op0=MULT, op1=ADD)
                nc.scalar.activation(out=h_t[:], in_=h_t[:],
                                     func=mybir.ActivationFunctionType.Relu)

                if h2 is None:
                    nc.sync.dma_start(out=hout[r0 : r0 + P, :], in_=h_t[:])
                else:
                    pt = ps.tile([P, P], F32, tag="tr")
                    nc.tensor.transpose(out=pt[:], in_=h_t[:], identity=ident[:])
                    hT = med.tile([P, P], F32, tag="hT")
                    nc.vector.tensor_copy(out=hT[:], in_=pt[:])
                    for nm, wnm, bnm in (("xl", "wl2", "bl2"), ("xr", "wr2", "br2"),
                                         ("skipb", "ws2", "bsk2")):
                        p2 = ps.tile([P, h2], F32, tag="mm")
                        nc.tensor.matmul(out=p2[:], lhsT=hT[:], rhs=w_t[wnm][:],
                                         start=True, stop=True)
                        ot = med.tile([P, h2], F32, tag="o_" + nm, name="o_" + nm)
                        nc.vector.tensor_tensor(out=ot[:], in0=p2[:], in1=b_t[bnm][:],
                                                op=ADD)
                        nc.sync.dma_start(out=outs[nm][r0 : r0 + P, :], in_=ot[:])
    nc.compile()
    return nc


# ----------------------------------------------------------------------------
# the kernel
# ----------------------------------------------------------------------------
def _run(nc, in_maps, n_cores):
    res = run_bass_kernel_spmd(nc, in_maps, core_ids=list(range(n_cores)), trace=TRACE)
    LAST_EXEC_NS.append(res.exec_time_ns)
    return res.results


def kernel(x, edge_index, Wl1, bl1, Wr1, br1, att1, bias1, Ws1, bs1,
           Wl2, bl2, Wr2, br2, att2, bias2, Ws2, bs2):
    global LAST_EXEC_NS
    LAST_EXEC_NS = []

    x = np.asarray(x, np.float32)
    to32 = lambda a: np.asarray(a, np.float32)
    Wl1, bl1, Wr1, br1, att1, bias1 = map(to32, (Wl1, bl1, Wr1, br1, att1, bias1))
    Ws1, bs1 = to32(Ws1), to32(bs1)
    Wl2, bl2, Wr2, br2, att2, bias2 = map(to32, (Wl2, bl2, Wr2, br2, att2, bias2))
    Ws2, bs2 = to32(Ws2), to32(bs2)

    meta = prep(edge_index)
    npc, nt, nv, Ks = meta["npc"], meta["nt"], meta["nv"], meta["Ks"]
    nodes_mat = meta["nodes_mat"]

    # per-core x slices, transposed (dummies -> zero columns)
    xsT = []
    for c in range(C):
        rows = nodes_mat[c]
        xs = np.zeros((npc, D_IN), np.float32)
        real = rows >= 0
        xs[real] = x[rows[real]]
        xsT.append(np.ascontiguousarray(xs.T))

    # ---- launch A: linear layer 1 -------------------------------------------
    nc_a = build_linear(npc, D_IN, HID)
    cb1 = bs1 + bias1
    in_a = [dict(xsT=xsT[c], wl=Wl1, wr=Wr1, ws=Ws1, bl=bl1, br=br1, bsk=cb1)
            for c in range(C)]
    res_a = _run(nc_a, in_a, C)

    xl1_full = np.empty((nv, HID), np.float32)
    for c in range(C):
        xl1_full[c * npc : (c + 1) * npc] = res_a[c]["o_xl"]
    xl1_full[-1] = 0.0

    # ---- launch BC: GAT layer 1 + linear layer 2 ----------------------------
    nc_bc = build_gat(npc, nv, Ks, HID, h2=OUT)
    cb2 = bs2 + bias2
    in_bc = [dict(xlf=xl1_full, xr=res_a[c]["o_xr"], skipb=res_a[c]["o_skipb"],
                  idx=meta["idx"][c], mask=meta["mask"][c], att=att1,
                  wl2=Wl2, wr2=Wr2, ws2=Ws2, bl2=bl2, br2=br2, bsk2=cb2)
             for c in range(C)]
    res_bc = _run(nc_bc, in_bc, C)

    xl2_full = np.empty((nv, OUT), np.float32)
    for c in range(C):
        xl2_full[c * npc : (c + 1) * npc] = res_bc[c]["o_xl"]
    xl2_full[-1] = 0.0

    # ---- launch D: GAT layer 2 ----------------------------------------------
    nc_d = build_gat(npc, nv, Ks, OUT, h2=None)
    in_d = [dict(xlf=xl2_full, xr=res_bc[c]["o_xr"], skipb=res_bc[c]["o_skipb"],
                 idx=meta["idx"][c], mask=meta["mask"][c], att=att2)
            for c in range(C)]
    res_d = _run(nc_d, in_d, C)

    out = np.empty((N_NODES, OUT), np.float32)
    nd = meta["n_dummy"]
    for c in range(C):
        out[nodes_mat[c, nd:]] = res_d[c]["o_h"][nd:]
    return out
